# revision 2
# baseline (speedup 1.0000x reference)
"""AttentionGuidedDynamicRangeDWConv3D on 8 Trainium2 NeuronCores.

Module: out = sum_i softmax(MLP(LN([mean_dhw(x), guidance])))[:, i]
                * dwconv3d(x, convw[i], convb[i], dil=i+1)
Shapes: x [4,96,16,56,56] f32, 3 branches of 3x3x3 depthwise conv with
dilations 1/2/3 ('same' zero padding).

Sharding: 8 cores = (batch b in 0..3) x (depth half h in 0..1). Each core
receives a host-padded 14-plane depth slab (global planes [8h-3, 8h+11),
out-of-range planes zero-filled) so every core runs the identical SPMD
program: owned output planes are always local planes [3, 11).

Layout per core: channels (96) on SBUF partitions, depth*H*W on the free
dim. Each of the 81 conv taps is one DVE scalar_tensor_tensor op
acc = x_shifted * w_eff[c] + acc, where w_eff folds the per-batch softmax
gate weights into the per-channel tap weights. 'same' padding in H/W is
handled by shrinking the access patterns at the borders (skipped border
reads contribute exactly zero). The gate MLP runs redundantly per core on
a [1,192] single-partition row; the global pooled features need a
cross-core 384-float AllReduce (each core contributes its half-batch
partial into its batch's column of a [96,4] buffer).
"""

import sys

if "/opt/trn_rl_repo" not in sys.path:
    sys.path.insert(0, "/opt/trn_rl_repo")

import numpy as np

import concourse.bass as bass
import concourse.mybir as mybir
import concourse.tile as tile
from concourse.bass_utils import run_bass_kernel_spmd

F32 = mybir.dt.float32
ALU = mybir.AluOpType
ACTF = mybir.ActivationFunctionType

B, C, D, H, W = 4, 96, 16, 56, 56
G, HID, NB = 96, 24, 3
K = 3
DILS = (1, 2, 3)
LN_EPS = 1e-5
N_CORES = 8
DH = D // 2          # planes per core (output)
NPL = DH + 2 * 3     # local input planes incl. 3-deep halo/zero pad
HW = H * W
PLANE = HW


def _tap_list():
    """[(tap_col, od, oh, ow)] with the full-AP center tap of branch 0 first."""
    taps = []
    for i, dil in enumerate(DILS):
        for kd in range(K):
            for kh in range(K):
                for kw in range(K):
                    t = i * 27 + kd * 9 + kh * 3 + kw
                    taps.append((t, (kd - 1) * dil, (kh - 1) * dil, (kw - 1) * dil))
    center = 0 * 27 + 1 * 9 + 1 * 3 + 1
    taps.sort(key=lambda e: e[0] != center)
    return taps


def _build_program():
    nc = bass.Bass()
    xin = nc.dram_tensor("x", [C, NPL * PLANE], F32, kind="ExternalInput")
    gdin = nc.dram_tensor("gd", [G], F32, kind="ExternalInput")
    cwt_in = nc.dram_tensor("cwt", [C, NB * 27], F32, kind="ExternalInput")
    cbt_in = nc.dram_tensor("cbt", [C, NB], F32, kind="ExternalInput")
    w1t_in = nc.dram_tensor("w1t", [HID, C + G], F32, kind="ExternalInput")
    b1_in = nc.dram_tensor("b1", [HID], F32, kind="ExternalInput")
    w2_in = nc.dram_tensor("w2", [HID, NB], F32, kind="ExternalInput")
    b2_in = nc.dram_tensor("b2", [NB], F32, kind="ExternalInput")
    lng_in = nc.dram_tensor("lng", [C + G], F32, kind="ExternalInput")
    lnb_in = nc.dram_tensor("lnb", [C + G], F32, kind="ExternalInput")
    oh4_in = nc.dram_tensor("oh4", [C, B], F32, kind="ExternalInput")
    yout = nc.dram_tensor("y", [C, DH * PLANE], F32, kind="ExternalOutput")

    with tile.TileContext(nc) as tc:
        with (
            tc.tile_pool(name="sbuf", bufs=1) as pool,
            tc.tile_pool(name="dram", bufs=1, space="DRAM") as dpool,
        ):
            xbuf = pool.tile([C, NPL * PLANE], F32, tag="xbuf")
            acc = pool.tile([C, PLANE], F32, tag="acc")
            w_eff = pool.tile([C, NB * 27], F32, tag="w_eff")
            cwt = pool.tile([C, NB * 27], F32, tag="cwt")
            cbt = pool.tile([C, NB], F32, tag="cbt")
            b_eff = pool.tile([C, 1], F32, tag="b_eff")
            tmpb = pool.tile([C, NB], F32, tag="tmpb")
            onehot_bc = pool.tile([C, B], F32, tag="onehot_bc")
            featp = pool.tile([C, 1], F32, tag="featp")
            contrib = pool.tile([C, B], F32, tag="contrib")
            ar_s = pool.tile([C, B], F32, tag="ar_s")
            feat_full = pool.tile([C, 1], F32, tag="feat_full")
            g_row = pool.tile([1, C + G], F32, tag="g_row")
            gd_row = pool.tile([1, C + G], F32, tag="gd_row")
            lng = pool.tile([1, C + G], F32, tag="lng")
            lnb = pool.tile([1, C + G], F32, tag="lnb")
            gn_row = pool.tile([1, C + G], F32, tag="gn_row")
            gn_bc = pool.tile([HID, C + G], F32, tag="gn_bc")
            w1t = pool.tile([HID, C + G], F32, tag="w1t")
            prod = pool.tile([HID, C + G], F32, tag="prod")
            hvec = pool.tile([HID, 1], F32, tag="hvec")
            b1c = pool.tile([HID, 1], F32, tag="b1c")
            w2t = pool.tile([HID, NB], F32, tag="w2t")
            l2tmp = pool.tile([HID, NB], F32, tag="l2tmp")
            z72 = pool.tile([1, HID * NB], F32, tag="z72")
            zrow = pool.tile([1, NB], F32, tag="zrow")
            b2r = pool.tile([1, NB], F32, tag="b2r")
            wts = pool.tile([1, NB], F32, tag="wts")
            wts_bc = pool.tile([C, NB], F32, tag="wts_bc")
            s1 = pool.tile([1, 1], F32, tag="s1")
            s2 = pool.tile([1, 1], F32, tag="s2")
            s3 = pool.tile([1, 1], F32, tag="s3")
            s4 = pool.tile([1, 1], F32, tag="s4")

            cin = dpool.tile([C, B], F32, tag="cin")
            cout = dpool.tile([C, B], F32, tag="cout")
            fb = dpool.tile([1, C], F32, tag="fb")
            zt = dpool.tile([1, HID * NB], F32, tag="zt")
            gb = dpool.tile([1, C + G], F32, tag="gb")
            wb = dpool.tile([1, NB], F32, tag="wb")

            v = nc.vector
            sc = nc.scalar

            # ---- loads ----
            nc.sync.dma_start(out=xbuf[:, :], in_=xin[:, :])
            nc.sync.dma_start(out=cwt[:, :], in_=cwt_in[:, :])
            nc.sync.dma_start(out=cbt[:, :], in_=cbt_in[:, :])
            nc.sync.dma_start(out=w1t[:, :], in_=w1t_in[:, :])
            nc.sync.dma_start(out=b1c[:, :], in_=b1_in[:, None])
            nc.sync.dma_start(out=w2t[:, :], in_=w2_in[:, :])
            nc.sync.dma_start(out=b2r[:, :], in_=b2_in[None, :])
            nc.sync.dma_start(out=lng[:, :], in_=lng_in[None, :])
            nc.sync.dma_start(out=lnb[:, :], in_=lnb_in[None, :])
            nc.sync.dma_start(out=onehot_bc[:, :], in_=oh4_in[:, :])
            nc.sync.dma_start(out=g_row[:, C:], in_=gdin[None, :])

            xv = xbuf[:, :].rearrange("c (d h w) -> c d h w", d=NPL, h=H, w=W)

            # ---- global-pool partial over owned planes [3, 3+DH) ----
            v.reduce_sum(featp[:, :], xv[:, 3 : 3 + DH], axis=mybir.AxisListType.XYZ)
            v.tensor_scalar_mul(featp[:, :], featp[:, :], 1.0 / (D * HW))
            v.tensor_scalar(
                out=contrib[:, :], in0=onehot_bc[:, :], scalar1=featp[:, :],
                scalar2=None, op0=ALU.mult,
            )

            # ---- cross-core AllReduce of [C, B] partials ----
            nc.sync.dma_start(out=cin[:, :], in_=contrib[:, :])
            nc.gpsimd.collective_compute(
                "AllReduce",
                ALU.add,
                replica_groups=[list(range(N_CORES))],
                ins=[cin.opt()],
                outs=[cout.opt()],
            )
            nc.sync.dma_start(out=ar_s[:, :], in_=cout[:, :])
            v.tensor_tensor(out=ar_s[:, :], in0=ar_s[:, :], in1=onehot_bc[:, :], op=ALU.mult)
            v.reduce_sum(feat_full[:, :], ar_s[:, :], axis=mybir.AxisListType.X)

            # ---- bounce feat to a single-partition row, build g=[feat|guidance]
            nc.sync.dma_start(out=fb[:, :], in_=feat_full[:, :])
            nc.sync.dma_start(out=g_row[:, :C], in_=fb[:, :])

            # ---- LayerNorm over 192 on one partition ----
            v.reduce_sum(s1[:, :], g_row[:, :], axis=mybir.AxisListType.X)
            v.tensor_scalar_mul(s1[:, :], s1[:, :], 1.0 / (C + G))  # mu
            v.tensor_scalar(
                out=gd_row[:, :], in0=g_row[:, :], scalar1=s1[:, :], scalar2=None,
                op0=ALU.subtract,
            )
            v.tensor_tensor(out=gn_row[:, :], in0=gd_row[:, :], in1=gd_row[:, :], op=ALU.mult)
            v.reduce_sum(s2[:, :], gn_row[:, :], axis=mybir.AxisListType.X)
            v.tensor_scalar(
                out=s2[:, :], in0=s2[:, :], scalar1=1.0 / (C + G), scalar2=LN_EPS,
                op0=ALU.mult, op1=ALU.add,
            )  # var + eps
            sc.activation(s3[:, :], s2[:, :], ACTF.Sqrt)
            # one Newton step: s4 = 0.5*(s3 + (var+eps)/s3) for a clean sqrt
            v.reciprocal(s4[:, :], s3[:, :])
            v.tensor_tensor(out=s4[:, :], in0=s4[:, :], in1=s2[:, :], op=ALU.mult)
            v.tensor_tensor(out=s4[:, :], in0=s4[:, :], in1=s3[:, :], op=ALU.add)
            v.tensor_scalar_mul(s4[:, :], s4[:, :], 0.5)
            v.reciprocal(s3[:, :], s4[:, :])  # rstd
            v.tensor_scalar(
                out=gn_row[:, :], in0=gd_row[:, :], scalar1=s3[:, :], scalar2=None,
                op0=ALU.mult,
            )
            v.tensor_tensor(out=gn_row[:, :], in0=gn_row[:, :], in1=lng[:, :], op=ALU.mult)
            v.tensor_tensor(out=gn_row[:, :], in0=gn_row[:, :], in1=lnb[:, :], op=ALU.add)

            # ---- MLP layer 1: h = gelu(gn @ w1 + b1) via row-products ----
            nc.sync.dma_start(out=gb[:, :], in_=gn_row[:, :])
            nc.sync.dma_start(out=gn_bc[:, :], in_=gb[:1, :].partition_broadcast(HID))
            v.tensor_tensor(out=prod[:, :], in0=w1t[:, :], in1=gn_bc[:, :], op=ALU.mult)
            v.reduce_sum(hvec[:, :], prod[:, :], axis=mybir.AxisListType.X)
            v.tensor_tensor(out=hvec[:, :], in0=hvec[:, :], in1=b1c[:, :], op=ALU.add)
            sc.activation(hvec[:, :], hvec[:, :], ACTF.Gelu)

            # ---- MLP layer 2 via DRAM transpose bounce ----
            v.tensor_scalar(
                out=l2tmp[:, :], in0=w2t[:, :], scalar1=hvec[:, :], scalar2=None,
                op0=ALU.mult,
            )
            nc.sync.dma_start(out=zt[:, :], in_=l2tmp[:, :])
            nc.sync.dma_start(out=z72[:, :], in_=zt[:, :])
            z3 = z72[:, :].rearrange("a (j i) -> a j i", j=HID, i=NB)
            for i in range(NB):
                v.reduce_sum(zrow[:, i : i + 1], z3[:, :, i], axis=mybir.AxisListType.X)
            v.tensor_tensor(out=zrow[:, :], in0=zrow[:, :], in1=b2r[:, :], op=ALU.add)

            # ---- softmax over 3 ----
            v.reduce_max(s1[:, :], zrow[:, :], axis=mybir.AxisListType.X)
            v.tensor_scalar(
                out=zrow[:, :], in0=zrow[:, :], scalar1=s1[:, :], scalar2=None,
                op0=ALU.subtract,
            )
            sc.activation(zrow[:, :], zrow[:, :], ACTF.Exp)
            v.reduce_sum(s2[:, :], zrow[:, :], axis=mybir.AxisListType.X)
            v.reciprocal(s2[:, :], s2[:, :])
            v.tensor_scalar(
                out=wts[:, :], in0=zrow[:, :], scalar1=s2[:, :], scalar2=None,
                op0=ALU.mult,
            )

            # ---- fold gate weights into per-tap channel weights ----
            nc.sync.dma_start(out=wb[:, :], in_=wts[:, :])
            nc.sync.dma_start(out=wts_bc[:, :], in_=wb[:1, :].partition_broadcast(C))
            for i in range(NB):
                v.tensor_scalar(
                    out=w_eff[:, i * 27 : (i + 1) * 27],
                    in0=cwt[:, i * 27 : (i + 1) * 27],
                    scalar1=wts_bc[:, i : i + 1],
                    scalar2=None,
                    op0=ALU.mult,
                )
            v.tensor_tensor(out=tmpb[:, :], in0=cbt[:, :], in1=wts_bc[:, :], op=ALU.mult)
            v.reduce_sum(b_eff[:, :], tmpb[:, :], axis=mybir.AxisListType.X)

            # ---- the conv: 81 fused MAC ops per output plane ----
            taps = _tap_list()
            accv = acc[:, :].rearrange("c (h w) -> c h w", h=H, w=W)
            for p in range(3, 3 + DH):
                for n, (t, od, oh, ow) in enumerate(taps):
                    h0i, h1i = max(0, oh), H + min(0, oh)
                    w0i, w1i = max(0, ow), W + min(0, ow)
                    h0o, h1o = max(0, -oh), H + min(0, -oh)
                    w0o, w1o = max(0, -ow), W + min(0, -ow)
                    in_ap = xv[:, p + od, h0i:h1i, w0i:w1i]
                    out_ap = accv[:, h0o:h1o, w0o:w1o]
                    if n == 0:
                        # full-extent center tap initializes acc with bias
                        v.tensor_scalar(
                            out=out_ap, in0=in_ap, scalar1=w_eff[:, t : t + 1],
                            scalar2=b_eff[:, :], op0=ALU.mult, op1=ALU.add,
                        )
                    else:
                        v.scalar_tensor_tensor(
                            out=out_ap, in0=in_ap, scalar=w_eff[:, t : t + 1],
                            in1=out_ap, op0=ALU.mult, op1=ALU.add,
                        )
                nc.sync.dma_start(
                    out=yout[:, (p - 3) * PLANE : (p - 2) * PLANE], in_=acc[:, :]
                )

    _split_sem_waits(nc)
    return nc


_WAITSPLIT = [0]


def _split_sem_waits(nc, max_waits=1):
    """This walrus build rejects >1 SyncWait per instruction (and any wait on
    a Drain). Move excess waits onto same-engine NOPs inserted just before."""
    for bb in nc.main_func.blocks:
        insns = bb.instructions
        i = 0
        while i < len(insns):
            ins = insns[i]
            si = ins.sync_info
            limit = 0 if ins.opcode == "Drain" else max_waits
            if si is not None and si.on_wait is not None and len(si.on_wait) > limit:
                waits = list(si.on_wait)
                keep = waits[-limit:] if limit else []
                extra = waits[: len(waits) - limit]
                pos = i
                for j in range(0, len(extra), max_waits):
                    nop = mybir.InstNoOp(
                        name=f"I-waitsplit-{_WAITSPLIT[0]}", ins=[], outs=[]
                    )
                    _WAITSPLIT[0] += 1
                    nop.engine = ins.engine
                    nop.sync_info = mybir.SyncInfo(
                        on_wait=extra[j : j + max_waits], on_update=[]
                    )
                    insns.insert(pos, nop)
                    pos += 1
                    i += 1
                si.on_wait = keep
            i += 1


def _prep_inputs(x, guidance, convw, convb, ln_g, ln_b, w1, b1, w2, b2):
    f = np.float32
    cwt = np.ascontiguousarray(
        convw.reshape(NB, C, 27).transpose(1, 0, 2).reshape(C, NB * 27), dtype=f
    )
    cbt = np.ascontiguousarray(convb.T, dtype=f)
    w1t = np.ascontiguousarray(w1.T, dtype=f)
    common = dict(
        cwt=cwt, cbt=cbt, w1t=w1t,
        b1=np.ascontiguousarray(b1, dtype=f),
        w2=np.ascontiguousarray(w2, dtype=f),
        b2=np.ascontiguousarray(b2, dtype=f),
        lng=np.ascontiguousarray(ln_g, dtype=f),
        lnb=np.ascontiguousarray(ln_b, dtype=f),
    )
    in_maps = []
    for core in range(N_CORES):
        b, h = core // 2, core % 2
        lo = 8 * h - 3
        shard = np.zeros((C, NPL, H, W), dtype=f)
        g0, g1 = max(0, lo), min(D, lo + NPL)
        shard[:, g0 - lo : g1 - lo] = x[b, :, g0:g1]
        onehot = np.zeros((C, B), dtype=f)
        onehot[:, b] = 1.0
        in_maps.append(
            dict(
                x=np.ascontiguousarray(shard.reshape(C, NPL * PLANE)),
                gd=np.ascontiguousarray(guidance[b], dtype=f),
                oh4=onehot,
                **common,
            )
        )
    return in_maps


_CACHED_NC = None


def kernel(x, guidance, convw, convb, ln_g, ln_b, w1, b1, w2, b2):
    global _CACHED_NC
    if _CACHED_NC is None:
        _CACHED_NC = _build_program()
    in_maps = _prep_inputs(
        x, guidance, convw, convb, ln_g, ln_b, w1, b1, w2, b2
    )
    res = run_bass_kernel_spmd(_CACHED_NC, in_maps, list(range(N_CORES)))
    out = np.empty((B, C, D, H, W), dtype=np.float32)
    for core in range(N_CORES):
        b, h = core // 2, core % 2
        out[b, :, 8 * h : 8 * h + 8] = res.results[core]["y"].reshape(C, DH, H, W)
    return out


if __name__ == "__main__":
    rng = np.random.default_rng(0)
    ins = dict(
        x=rng.standard_normal((B, C, D, H, W), dtype=np.float32),
        guidance=rng.standard_normal((B, G), dtype=np.float32),
        convw=(rng.standard_normal((NB, C, 1, K, K, K)) * 0.1).astype(np.float32),
        convb=np.zeros((NB, C), np.float32),
        ln_g=np.ones((C + G,), np.float32),
        ln_b=np.zeros((C + G,), np.float32),
        w1=(rng.standard_normal((C + G, HID)) * 0.05).astype(np.float32),
        b1=np.zeros((HID,), np.float32),
        w2=(rng.standard_normal((HID, NB)) * 0.05).astype(np.float32),
        b2=np.zeros((NB,), np.float32),
    )
    out = kernel(**ins)
    print("kernel ran, out shape", out.shape, "mean", float(np.abs(out).mean()))


# revision 6
# speedup vs baseline: 2.6181x; 2.6181x over previous
"""AttentionGuidedDynamicRangeDWConv3D on 8 Trainium2 NeuronCores.

Module: out = sum_i softmax(MLP(LN([mean_dhw(x), guidance])))[:, i]
                * dwconv3d(x, convw[i], convb[i], dil=i+1)
Shapes: x [4,96,16,56,56] f32, 3 branches of 3x3x3 depthwise conv with
dilations 1/2/3 ('same' zero padding).

Sharding: 8 cores = (batch b in 0..3) x (depth half h in 0..1). Each core
receives a host-padded 14-plane depth slab (global planes [8h-3, 8h+11),
out-of-range planes zero-filled) so every core runs the identical SPMD
program: owned output planes are always local planes [3, 11).

Layout per core: channels (96) on SBUF partitions, depth*H*W on the free
dim. The 81 conv taps are split between two engines working in parallel:

- VectorE: fused MACs acc = x_shifted * w_eff[c] + acc
  (scalar_tensor_tensor with a per-partition [96,1] weight column).
  'same' padding in H/W via shrunken access patterns.
- TensorE: per-tap diagonal matmuls diag(w_eff[:,t]) @ x_shifted
  accumulated in PSUM, using the 4x-faster fp32r mode (x is
  host-pre-rounded to fp32r's 11-bit mantissa; weights rounded on chip).
  fp32r matmuls require flat contiguous operands, so taps are applied as
  flat shifts over 448-column PSUM chunks; plane-edge chunks are trimmed
  for the h-shift, and the w-shift wraparound columns (which a flat shift
  gets wrong) are fixed up afterwards by small VectorE subtract ops.
  The diagonal weight tiles are rebuilt per (plane, tap) by the otherwise
  idle ScalarE into a 4-slot rotating pool (ScalarE also pre-zeroes the
  PSUM chunks, removing any matmul-accumulation start-flag hazards).

w_eff folds the per-batch softmax gate weights into the per-channel tap
weights. The gate MLP runs redundantly per core on a [1,192] row; the
global pooled features need one cross-core 384-float AllReduce.
"""

import sys

if "/opt/trn_rl_repo" not in sys.path:
    sys.path.insert(0, "/opt/trn_rl_repo")

import numpy as np

import concourse.bass as bass
import concourse.mybir as mybir
import concourse.tile as tile
from concourse.bass_utils import run_bass_kernel_spmd

F32 = mybir.dt.float32
F32R = mybir.dt.float32r
ALU = mybir.AluOpType
ACTF = mybir.ActivationFunctionType

B, C, D, H, W = 4, 96, 16, 56, 56
G, HID, NB = 96, 24, 3
K = 3
DILS = (1, 2, 3)
LN_EPS = 1e-5
N_CORES = 8
DVE_TAPS = 23        # taps computed on DVE; the rest go to the PE
CHUNK = 448          # PSUM chunk: 8 h-rows of one plane
N_CHUNKS = 7
XG = 16              # front guard elems for flat-shifted PE reads
XGB = 96             # back guard (fix-up row-slices can overrun the data end)
DH = D // 2          # planes per core (output)
NPL = DH + 2 * 3     # local input planes incl. 3-deep halo/zero pad
HW = H * W
PLANE = HW


def _tap_list():
    """[(tap_col, od, oh, ow)]; center tap of branch 0 first (it initializes
    acc with the bias), then the other ow!=0 taps (DVE side prefers those:
    each PE ow!=0 tap costs an extra wrap fix-up op)."""
    taps = []
    for i, dil in enumerate(DILS):
        for kd in range(K):
            for kh in range(K):
                for kw in range(K):
                    t = i * 27 + kd * 9 + kh * 3 + kw
                    taps.append((t, (kd - 1) * dil, (kh - 1) * dil, (kw - 1) * dil))
    center = 0 * 27 + 1 * 9 + 1 * 3 + 1
    ctr = next(e for e in taps if e[0] == center)
    rest = [e for e in taps if e[0] != center]
    rest.sort(key=lambda e: e[3] == 0)
    return [ctr] + rest


def _build_program():
    nc = bass.Bass()
    xin = nc.dram_tensor("x", [C, NPL * PLANE], F32R, kind="ExternalInput")
    gdin = nc.dram_tensor("gd", [G], F32, kind="ExternalInput")
    cwt_in = nc.dram_tensor("cwt", [C, NB * 27], F32, kind="ExternalInput")
    cbt_in = nc.dram_tensor("cbt", [C, NB], F32, kind="ExternalInput")
    w1t_in = nc.dram_tensor("w1t", [HID, C + G], F32, kind="ExternalInput")
    b1_in = nc.dram_tensor("b1", [HID], F32, kind="ExternalInput")
    w2_in = nc.dram_tensor("w2", [HID, NB], F32, kind="ExternalInput")
    b2_in = nc.dram_tensor("b2", [NB], F32, kind="ExternalInput")
    lng_in = nc.dram_tensor("lng", [C + G], F32, kind="ExternalInput")
    lnb_in = nc.dram_tensor("lnb", [C + G], F32, kind="ExternalInput")
    oh4_in = nc.dram_tensor("oh4", [C, B], F32, kind="ExternalInput")
    id_in = nc.dram_tensor("idp", [C, C], F32, kind="ExternalInput")
    yout = nc.dram_tensor("y", [C, DH * PLANE], F32, kind="ExternalOutput")

    with tile.TileContext(nc) as tc:
        with (
            tc.tile_pool(name="sbuf", bufs=1) as pool,
            tc.tile_pool(name="diagp", bufs=4) as diagpool,
            tc.tile_pool(name="dram", bufs=1, space="DRAM") as dpool,
            tc.tile_pool(name="psum", bufs=1, space="PSUM") as ppool,
        ):
            xbuf = pool.tile([C, XG + NPL * PLANE + XGB], F32R, tag="xbuf")
            acc = pool.tile([C, PLANE], F32, tag="acc")
            w_eff = pool.tile([C, NB * 27], F32, tag="w_eff")
            w_neg = pool.tile([C, NB * 27], F32, tag="w_neg")
            cwt = pool.tile([C, NB * 27], F32, tag="cwt")
            cbt = pool.tile([C, NB], F32, tag="cbt")
            b_eff = pool.tile([C, 1], F32, tag="b_eff")
            tmpb = pool.tile([C, NB], F32, tag="tmpb")
            onehot_bc = pool.tile([C, B], F32, tag="onehot_bc")
            featp = pool.tile([C, 1], F32, tag="featp")
            contrib = pool.tile([C, B], F32, tag="contrib")
            ar_s = pool.tile([C, B], F32, tag="ar_s")
            feat_full = pool.tile([C, 1], F32, tag="feat_full")
            g_row = pool.tile([1, C + G], F32, tag="g_row")
            gd_row = pool.tile([1, C + G], F32, tag="gd_row")
            lng = pool.tile([1, C + G], F32, tag="lng")
            lnb = pool.tile([1, C + G], F32, tag="lnb")
            gn_row = pool.tile([1, C + G], F32, tag="gn_row")
            gn_bc = pool.tile([HID, C + G], F32, tag="gn_bc")
            w1t = pool.tile([HID, C + G], F32, tag="w1t")
            prod = pool.tile([HID, C + G], F32, tag="prod")
            hvec = pool.tile([HID, 1], F32, tag="hvec")
            b1c = pool.tile([HID, 1], F32, tag="b1c")
            w2t = pool.tile([HID, NB], F32, tag="w2t")
            l2tmp = pool.tile([HID, NB], F32, tag="l2tmp")
            z72 = pool.tile([1, HID * NB], F32, tag="z72")
            zrow = pool.tile([1, NB], F32, tag="zrow")
            b2r = pool.tile([1, NB], F32, tag="b2r")
            wts = pool.tile([1, NB], F32, tag="wts")
            wts_bc = pool.tile([C, NB], F32, tag="wts_bc")
            idp = pool.tile([C, C], F32, tag="idp")
            s1 = pool.tile([1, 1], F32, tag="s1")
            s2 = pool.tile([1, 1], F32, tag="s2")
            s3 = pool.tile([1, 1], F32, tag="s3")
            s4 = pool.tile([1, 1], F32, tag="s4")

            cin = dpool.tile([C, B], F32, tag="cin")
            cout = dpool.tile([C, B], F32, tag="cout")
            fb = dpool.tile([1, C], F32, tag="fb")
            zt = dpool.tile([1, HID * NB], F32, tag="zt")
            gb = dpool.tile([1, C + G], F32, tag="gb")
            wb = dpool.tile([1, NB], F32, tag="wb")

            v = nc.vector
            sc = nc.scalar

            # ---- loads ----
            nc.sync.dma_start(out=xbuf[:, XG : XG + NPL * PLANE], in_=xin[:, :])
            nc.sync.dma_start(out=cwt[:, :], in_=cwt_in[:, :])
            nc.sync.dma_start(out=cbt[:, :], in_=cbt_in[:, :])
            nc.sync.dma_start(out=w1t[:, :], in_=w1t_in[:, :])
            nc.sync.dma_start(out=b1c[:, :], in_=b1_in[:, None])
            nc.sync.dma_start(out=w2t[:, :], in_=w2_in[:, :])
            nc.sync.dma_start(out=b2r[:, :], in_=b2_in[None, :])
            nc.sync.dma_start(out=lng[:, :], in_=lng_in[None, :])
            nc.sync.dma_start(out=lnb[:, :], in_=lnb_in[None, :])
            nc.sync.dma_start(out=onehot_bc[:, :], in_=oh4_in[:, :])
            nc.sync.dma_start(out=idp[:, :], in_=id_in[:, :])
            nc.sync.dma_start(out=g_row[:, C:], in_=gdin[None, :])

            xflat_r = xbuf[:, :]                       # fp32r view (PE rhs)
            xflat = xbuf[:, :].bitcast(F32)            # f32 view (DVE)
            xv = xflat[:, XG : XG + NPL * PLANE].rearrange(
                "c (d h w) -> c d h w", d=NPL, h=H, w=W
            )

            # ---- global-pool partial over owned planes [3, 3+DH) ----
            v.reduce_sum(featp[:, :], xv[:, 3 : 3 + DH], axis=mybir.AxisListType.XYZ)
            v.tensor_scalar_mul(featp[:, :], featp[:, :], 1.0 / (D * HW))
            v.tensor_scalar(
                out=contrib[:, :], in0=onehot_bc[:, :], scalar1=featp[:, :],
                scalar2=None, op0=ALU.mult,
            )

            # ---- cross-core AllReduce of [C, B] partials ----
            nc.sync.dma_start(out=cin[:, :], in_=contrib[:, :])
            nc.gpsimd.collective_compute(
                "AllReduce",
                ALU.add,
                replica_groups=[list(range(N_CORES))],
                ins=[cin.opt()],
                outs=[cout.opt()],
            )
            nc.sync.dma_start(out=ar_s[:, :], in_=cout[:, :])
            v.tensor_tensor(out=ar_s[:, :], in0=ar_s[:, :], in1=onehot_bc[:, :], op=ALU.mult)
            v.reduce_sum(feat_full[:, :], ar_s[:, :], axis=mybir.AxisListType.X)

            # ---- bounce feat to a single-partition row, build g=[feat|guidance]
            nc.sync.dma_start(out=fb[:, :], in_=feat_full[:, :])
            nc.sync.dma_start(out=g_row[:, :C], in_=fb[:, :])

            # ---- LayerNorm over 192 on one partition ----
            v.reduce_sum(s1[:, :], g_row[:, :], axis=mybir.AxisListType.X)
            v.tensor_scalar_mul(s1[:, :], s1[:, :], 1.0 / (C + G))  # mu
            v.tensor_scalar(
                out=gd_row[:, :], in0=g_row[:, :], scalar1=s1[:, :], scalar2=None,
                op0=ALU.subtract,
            )
            v.tensor_tensor(out=gn_row[:, :], in0=gd_row[:, :], in1=gd_row[:, :], op=ALU.mult)
            v.reduce_sum(s2[:, :], gn_row[:, :], axis=mybir.AxisListType.X)
            v.tensor_scalar(
                out=s2[:, :], in0=s2[:, :], scalar1=1.0 / (C + G), scalar2=LN_EPS,
                op0=ALU.mult, op1=ALU.add,
            )  # var + eps
            sc.activation(s3[:, :], s2[:, :], ACTF.Sqrt)
            # one Newton step: s4 = 0.5*(s3 + (var+eps)/s3) for a clean sqrt
            v.reciprocal(s4[:, :], s3[:, :])
            v.tensor_tensor(out=s4[:, :], in0=s4[:, :], in1=s2[:, :], op=ALU.mult)
            v.tensor_tensor(out=s4[:, :], in0=s4[:, :], in1=s3[:, :], op=ALU.add)
            v.tensor_scalar_mul(s4[:, :], s4[:, :], 0.5)
            v.reciprocal(s3[:, :], s4[:, :])  # rstd
            v.tensor_scalar(
                out=gn_row[:, :], in0=gd_row[:, :], scalar1=s3[:, :], scalar2=None,
                op0=ALU.mult,
            )
            v.tensor_tensor(out=gn_row[:, :], in0=gn_row[:, :], in1=lng[:, :], op=ALU.mult)
            v.tensor_tensor(out=gn_row[:, :], in0=gn_row[:, :], in1=lnb[:, :], op=ALU.add)

            # ---- MLP layer 1: h = gelu(gn @ w1 + b1) via row-products ----
            nc.sync.dma_start(out=gb[:, :], in_=gn_row[:, :])
            nc.sync.dma_start(out=gn_bc[:, :], in_=gb[:1, :].partition_broadcast(HID))
            v.tensor_tensor(out=prod[:, :], in0=w1t[:, :], in1=gn_bc[:, :], op=ALU.mult)
            v.reduce_sum(hvec[:, :], prod[:, :], axis=mybir.AxisListType.X)
            v.tensor_tensor(out=hvec[:, :], in0=hvec[:, :], in1=b1c[:, :], op=ALU.add)
            sc.activation(hvec[:, :], hvec[:, :], ACTF.Gelu)

            # ---- MLP layer 2 via DRAM transpose bounce ----
            v.tensor_scalar(
                out=l2tmp[:, :], in0=w2t[:, :], scalar1=hvec[:, :], scalar2=None,
                op0=ALU.mult,
            )
            nc.sync.dma_start(out=zt[:, :], in_=l2tmp[:, :])
            nc.sync.dma_start(out=z72[:, :], in_=zt[:, :])
            z3 = z72[:, :].rearrange("a (j i) -> a j i", j=HID, i=NB)
            for i in range(NB):
                v.reduce_sum(zrow[:, i : i + 1], z3[:, :, i], axis=mybir.AxisListType.X)
            v.tensor_tensor(out=zrow[:, :], in0=zrow[:, :], in1=b2r[:, :], op=ALU.add)

            # ---- softmax over 3 ----
            v.reduce_max(s1[:, :], zrow[:, :], axis=mybir.AxisListType.X)
            v.tensor_scalar(
                out=zrow[:, :], in0=zrow[:, :], scalar1=s1[:, :], scalar2=None,
                op0=ALU.subtract,
            )
            sc.activation(zrow[:, :], zrow[:, :], ACTF.Exp)
            v.reduce_sum(s2[:, :], zrow[:, :], axis=mybir.AxisListType.X)
            v.reciprocal(s2[:, :], s2[:, :])
            v.tensor_scalar(
                out=wts[:, :], in0=zrow[:, :], scalar1=s2[:, :], scalar2=None,
                op0=ALU.mult,
            )

            # ---- fold gate weights into per-tap channel weights ----
            nc.sync.dma_start(out=wb[:, :], in_=wts[:, :])
            nc.sync.dma_start(out=wts_bc[:, :], in_=wb[:1, :].partition_broadcast(C))
            for i in range(NB):
                v.tensor_scalar(
                    out=w_eff[:, i * 27 : (i + 1) * 27],
                    in0=cwt[:, i * 27 : (i + 1) * 27],
                    scalar1=wts_bc[:, i : i + 1],
                    scalar2=None,
                    op0=ALU.mult,
                )
            v.tensor_scalar_mul(w_neg[:, :], w_eff[:, :], -1.0)
            v.tensor_tensor(out=tmpb[:, :], in0=cbt[:, :], in1=wts_bc[:, :], op=ALU.mult)
            v.reduce_sum(b_eff[:, :], tmpb[:, :], axis=mybir.AxisListType.X)

            # ---- the conv ----
            taps = _tap_list()
            dve_taps = taps[:DVE_TAPS]
            pe_taps = taps[DVE_TAPS:]
            accv = acc[:, :].rearrange("c (h w) -> c h w", h=H, w=W)
            for p in range(3, 3 + DH):
                # PE side: ScalarE zeroes psum chunks and rebuilds each tap's
                # diagonal; TensorE runs flat fp32r matmuls per 448-col chunk.
                pss = []
                for ci in range(N_CHUNKS):
                    ps = ppool.tile([C, CHUNK], F32, tag=f"ps{ci}")
                    sc.memzero(ps[:, :])
                    pss.append(ps)
                for t, od, oh, ow in pe_taps:
                    dg = diagpool.tile([C, C], F32R, tag="diag")
                    sc.activation(dg[:, :], idp[:, :], ACTF.Copy,
                                  scale=w_eff[:, t : t + 1])
                    lo_trim = max(0, -oh) * W
                    hi_trim = (H - max(0, oh)) * W
                    for ci in range(N_CHUNKS):
                        a = max(ci * CHUNK, lo_trim)
                        b = min((ci + 1) * CHUNK, hi_trim)
                        src0 = XG + (p + od) * PLANE + a + oh * W + ow
                        nc.tensor.matmul(
                            pss[ci][:, a - ci * CHUNK : b - ci * CHUNK],
                            dg[:, :],
                            xflat_r[:, src0 : src0 + (b - a)],
                            start=False,
                            stop=False,
                            skip_group_check=True,
                        )
                # DVE side: exact fp32 MACs (on the rounded x)
                for n, (t, od, oh, ow) in enumerate(dve_taps):
                    h0i, h1i = max(0, oh), H + min(0, oh)
                    w0i, w1i = max(0, ow), W + min(0, ow)
                    h0o, h1o = max(0, -oh), H + min(0, -oh)
                    w0o, w1o = max(0, -ow), W + min(0, -ow)
                    in_ap = xv[:, p + od, h0i:h1i, w0i:w1i]
                    out_ap = accv[:, h0o:h1o, w0o:w1o]
                    if n == 0:
                        # full-extent center tap initializes acc with bias
                        v.tensor_scalar(
                            out=out_ap, in0=in_ap, scalar1=w_eff[:, t : t + 1],
                            scalar2=b_eff[:, :], op0=ALU.mult, op1=ALU.add,
                        )
                    else:
                        v.scalar_tensor_tensor(
                            out=out_ap, in0=in_ap, scalar=w_eff[:, t : t + 1],
                            in1=out_ap, op0=ALU.mult, op1=ALU.add,
                        )
                # merge PSUM chunks into acc
                for ci in range(N_CHUNKS):
                    seg = slice(ci * CHUNK, (ci + 1) * CHUNK)
                    v.tensor_tensor(
                        out=acc[:, seg], in0=acc[:, seg], in1=pss[ci][:, :],
                        op=ALU.add,
                    )
                # fix up the w-wrap columns the flat PE shifts got wrong
                for t, od, oh, ow in pe_taps:
                    if ow == 0:
                        continue
                    r0 = max(0, -oh)
                    nr = H - abs(oh)
                    w0 = W - ow if ow > 0 else 0
                    nw = abs(ow)
                    base = XG + (p + od) * PLANE + (r0 + oh) * W + (w0 + ow)
                    src = xflat[:, base : base + nr * W].rearrange(
                        "c (r w) -> c r w", r=nr, w=W
                    )[:, :, 0:nw]
                    out2d = accv[:, r0 : r0 + nr, w0 : w0 + nw]
                    v.scalar_tensor_tensor(
                        out=out2d, in0=src, scalar=w_neg[:, t : t + 1],
                        in1=out2d, op0=ALU.mult, op1=ALU.add,
                    )
                nc.sync.dma_start(
                    out=yout[:, (p - 3) * PLANE : (p - 2) * PLANE], in_=acc[:, :]
                )

    _split_sem_waits(nc)
    return nc


_WAITSPLIT = [0]


def _split_sem_waits(nc, max_waits=1):
    """This walrus build rejects >1 SyncWait per instruction (and any wait on
    a Drain). Move excess waits onto same-engine NOPs inserted just before."""
    for bb in nc.main_func.blocks:
        insns = bb.instructions
        i = 0
        while i < len(insns):
            ins = insns[i]
            si = ins.sync_info
            limit = 0 if ins.opcode == "Drain" else max_waits
            if si is not None and si.on_wait is not None and len(si.on_wait) > limit:
                waits = list(si.on_wait)
                keep = waits[-limit:] if limit else []
                extra = waits[: len(waits) - limit]
                pos = i
                for j in range(0, len(extra), max_waits):
                    nop = mybir.InstNoOp(
                        name=f"I-waitsplit-{_WAITSPLIT[0]}", ins=[], outs=[]
                    )
                    _WAITSPLIT[0] += 1
                    nop.engine = ins.engine
                    nop.sync_info = mybir.SyncInfo(
                        on_wait=extra[j : j + max_waits], on_update=[]
                    )
                    insns.insert(pos, nop)
                    pos += 1
                    i += 1
                si.on_wait = keep
            i += 1


def _round_fp32r(a):
    u = np.ascontiguousarray(a, dtype=np.float32).view(np.uint32)
    lsb = (u >> 12) & 1
    r = ((u + 0x7FF + lsb) & np.uint32(0xFFFFF000)).astype(np.uint32)
    return r.view(np.float32)


def _prep_inputs(x, guidance, convw, convb, ln_g, ln_b, w1, b1, w2, b2):
    f = np.float32
    cwt = np.ascontiguousarray(
        convw.reshape(NB, C, 27).transpose(1, 0, 2).reshape(C, NB * 27), dtype=f
    )
    cbt = np.ascontiguousarray(convb.T, dtype=f)
    w1t = np.ascontiguousarray(w1.T, dtype=f)
    idp = np.eye(C, dtype=f)
    common = dict(
        cwt=cwt, cbt=cbt, w1t=w1t,
        b1=np.ascontiguousarray(b1, dtype=f),
        w2=np.ascontiguousarray(w2, dtype=f),
        b2=np.ascontiguousarray(b2, dtype=f),
        lng=np.ascontiguousarray(ln_g, dtype=f),
        lnb=np.ascontiguousarray(ln_b, dtype=f),
        idp=idp,
    )
    in_maps = []
    for core in range(N_CORES):
        b, h = core // 2, core % 2
        lo = 8 * h - 3
        shard = np.zeros((C, NPL, H, W), dtype=f)
        g0, g1 = max(0, lo), min(D, lo + NPL)
        shard[:, g0 - lo : g1 - lo] = x[b, :, g0:g1]
        onehot = np.zeros((C, B), dtype=f)
        onehot[:, b] = 1.0
        in_maps.append(
            dict(
                x=_round_fp32r(shard.reshape(C, NPL * PLANE)),
                gd=np.ascontiguousarray(guidance[b], dtype=f),
                oh4=onehot,
                **common,
            )
        )
    return in_maps


_CACHED_NC = None


def kernel(x, guidance, convw, convb, ln_g, ln_b, w1, b1, w2, b2):
    global _CACHED_NC
    if _CACHED_NC is None:
        _CACHED_NC = _build_program()
    in_maps = _prep_inputs(
        x, guidance, convw, convb, ln_g, ln_b, w1, b1, w2, b2
    )
    res = run_bass_kernel_spmd(_CACHED_NC, in_maps, list(range(N_CORES)))
    out = np.empty((B, C, D, H, W), dtype=np.float32)
    for core in range(N_CORES):
        b, h = core // 2, core % 2
        out[b, :, 8 * h : 8 * h + 8] = res.results[core]["y"].reshape(C, DH, H, W)
    return out


if __name__ == "__main__":
    rng = np.random.default_rng(0)
    ins = dict(
        x=rng.standard_normal((B, C, D, H, W), dtype=np.float32),
        guidance=rng.standard_normal((B, G), dtype=np.float32),
        convw=(rng.standard_normal((NB, C, 1, K, K, K)) * 0.1).astype(np.float32),
        convb=np.zeros((NB, C), np.float32),
        ln_g=np.ones((C + G,), np.float32),
        ln_b=np.zeros((C + G,), np.float32),
        w1=(rng.standard_normal((C + G, HID)) * 0.05).astype(np.float32),
        b1=np.zeros((HID,), np.float32),
        w2=(rng.standard_normal((HID, NB)) * 0.05).astype(np.float32),
        b2=np.zeros((NB,), np.float32),
    )
    out = kernel(**ins)
    print("kernel ran, out shape", out.shape, "mean", float(np.abs(out).mean()))


# revision 9
# speedup vs baseline: 2.9387x; 1.1224x over previous
"""AttentionGuidedDynamicRangeDWConv3D on 8 Trainium2 NeuronCores.

Module: out = sum_i softmax(MLP(LN([mean_dhw(x), guidance])))[:, i]
                * dwconv3d(x, convw[i], convb[i], dil=i+1)
Shapes: x [4,96,16,56,56] f32, 3 branches of 3x3x3 depthwise conv with
dilations 1/2/3 ('same' zero padding).

Sharding: 8 cores = (batch b in 0..3) x (depth half h in 0..1). Each core
receives a host-padded 14-plane depth slab (global planes [8h-3, 8h+11),
out-of-range planes zero-filled) so every core runs the identical SPMD
program: owned output planes are always local planes [3, 11).

Layout per core: channels (96) on SBUF partitions, depth*H*W on the free
dim. The 81 conv taps are split between two engines working in parallel:

- VectorE: fused MACs acc = x_shifted * w_eff[c] + acc
  (scalar_tensor_tensor with a per-partition [96,1] weight column).
  'same' padding in H/W via shrunken access patterns.
- TensorE: per-tap diagonal matmuls diag(w_eff[:,t]) @ x_shifted
  accumulated in PSUM, using the 4x-faster fp32r mode (x is
  host-pre-rounded to fp32r's 11-bit mantissa; weights rounded on chip).
  fp32r matmuls require flat contiguous operands, so taps are applied as
  flat shifts over 448-column PSUM chunks; plane-edge chunks are trimmed
  for the h-shift, and the w-shift wraparound columns (which a flat shift
  gets wrong) are fixed up afterwards by small VectorE subtract ops.
  The diagonal weight tiles are rebuilt per (plane, tap) by the otherwise
  idle ScalarE into a 4-slot rotating pool (ScalarE also pre-zeroes the
  PSUM chunks, removing any matmul-accumulation start-flag hazards).

w_eff folds the per-batch softmax gate weights into the per-channel tap
weights. The gate MLP runs redundantly per core on a [1,192] row; the
global pooled features need one cross-core 384-float AllReduce.
"""

import sys

if "/opt/trn_rl_repo" not in sys.path:
    sys.path.insert(0, "/opt/trn_rl_repo")

import numpy as np

import concourse.bass as bass
import concourse.mybir as mybir
import concourse.tile as tile
from concourse.bass_utils import run_bass_kernel_spmd

F32 = mybir.dt.float32
F32R = mybir.dt.float32r
ALU = mybir.AluOpType
ACTF = mybir.ActivationFunctionType

B, C, D, H, W = 4, 96, 16, 56, 56
G, HID, NB = 96, 24, 3
K = 3
DILS = (1, 2, 3)
LN_EPS = 1e-5
N_CORES = 8
DVE_TAPS = 19        # taps computed on DVE
GP_TAPS = 0          # GPSIMD rejected by this walrus (Pool engine check)
CHUNK = 448          # PSUM chunk: 8 h-rows of one plane
N_CHUNKS = 7
XG = 16              # front guard elems for flat-shifted PE reads
XGB = 96             # back guard (fix-up row-slices can overrun the data end)
DH = D // 2          # planes per core (output)
NPL = DH + 2 * 3     # local input planes incl. 3-deep halo/zero pad
HW = H * W
PLANE = HW


def _tap_list():
    """[(tap_col, od, oh, ow)]; center tap of branch 0 first (it initializes
    acc with the bias), then the other ow!=0 taps (DVE side prefers those:
    each PE ow!=0 tap costs an extra wrap fix-up op)."""
    taps = []
    for i, dil in enumerate(DILS):
        for kd in range(K):
            for kh in range(K):
                for kw in range(K):
                    t = i * 27 + kd * 9 + kh * 3 + kw
                    taps.append((t, (kd - 1) * dil, (kh - 1) * dil, (kw - 1) * dil))
    center = 0 * 27 + 1 * 9 + 1 * 3 + 1
    ctr = next(e for e in taps if e[0] == center)
    rest = [e for e in taps if e[0] != center]
    rest.sort(key=lambda e: e[3] == 0)
    return [ctr] + rest


def _build_program():
    nc = bass.Bass()
    xin = nc.dram_tensor("x", [C, NPL * PLANE], F32R, kind="ExternalInput")
    gdin = nc.dram_tensor("gd", [G], F32, kind="ExternalInput")
    cwt_in = nc.dram_tensor("cwt", [C, NB * 27], F32, kind="ExternalInput")
    cbt_in = nc.dram_tensor("cbt", [C, NB], F32, kind="ExternalInput")
    w1t_in = nc.dram_tensor("w1t", [HID, C + G], F32, kind="ExternalInput")
    b1_in = nc.dram_tensor("b1", [HID], F32, kind="ExternalInput")
    w2_in = nc.dram_tensor("w2", [HID, NB], F32, kind="ExternalInput")
    b2_in = nc.dram_tensor("b2", [NB], F32, kind="ExternalInput")
    lng_in = nc.dram_tensor("lng", [C + G], F32, kind="ExternalInput")
    lnb_in = nc.dram_tensor("lnb", [C + G], F32, kind="ExternalInput")
    oh4_in = nc.dram_tensor("oh4", [C, B], F32, kind="ExternalInput")
    id_in = nc.dram_tensor("idp", [C, C], F32, kind="ExternalInput")
    yout = nc.dram_tensor("y", [C, DH * PLANE], F32, kind="ExternalOutput")

    with tile.TileContext(nc) as tc:
        with (
            tc.tile_pool(name="sbuf", bufs=1) as pool,
            tc.tile_pool(name="diagp", bufs=4) as diagpool,
            tc.tile_pool(name="dram", bufs=1, space="DRAM") as dpool,
            tc.tile_pool(name="psum", bufs=1, space="PSUM") as ppool,
        ):
            xbuf = pool.tile([C, XG + NPL * PLANE + XGB], F32R, tag="xbuf")
            acc = pool.tile([C, PLANE], F32, tag="acc")
            acc2 = pool.tile([C, PLANE], F32, tag="acc2")
            w_eff = pool.tile([C, NB * 27], F32, tag="w_eff")
            w_neg = pool.tile([C, NB * 27], F32, tag="w_neg")
            cwt = pool.tile([C, NB * 27], F32, tag="cwt")
            cbt = pool.tile([C, NB], F32, tag="cbt")
            b_eff = pool.tile([C, 1], F32, tag="b_eff")
            tmpb = pool.tile([C, NB], F32, tag="tmpb")
            onehot_bc = pool.tile([C, B], F32, tag="onehot_bc")
            featp = pool.tile([C, 1], F32, tag="featp")
            contrib = pool.tile([C, B], F32, tag="contrib")
            ar_s = pool.tile([C, B], F32, tag="ar_s")
            feat_full = pool.tile([C, 1], F32, tag="feat_full")
            g_row = pool.tile([1, C + G], F32, tag="g_row")
            gd_row = pool.tile([1, C + G], F32, tag="gd_row")
            lng = pool.tile([1, C + G], F32, tag="lng")
            lnb = pool.tile([1, C + G], F32, tag="lnb")
            gn_row = pool.tile([1, C + G], F32, tag="gn_row")
            gn_bc = pool.tile([HID, C + G], F32, tag="gn_bc")
            w1t = pool.tile([HID, C + G], F32, tag="w1t")
            prod = pool.tile([HID, C + G], F32, tag="prod")
            hvec = pool.tile([HID, 1], F32, tag="hvec")
            b1c = pool.tile([HID, 1], F32, tag="b1c")
            w2t = pool.tile([HID, NB], F32, tag="w2t")
            l2tmp = pool.tile([HID, NB], F32, tag="l2tmp")
            z72 = pool.tile([1, HID * NB], F32, tag="z72")
            zrow = pool.tile([1, NB], F32, tag="zrow")
            b2r = pool.tile([1, NB], F32, tag="b2r")
            wts = pool.tile([1, NB], F32, tag="wts")
            wts_bc = pool.tile([C, NB], F32, tag="wts_bc")
            idp = pool.tile([C, C], F32, tag="idp")
            s1 = pool.tile([1, 1], F32, tag="s1")
            s2 = pool.tile([1, 1], F32, tag="s2")
            s3 = pool.tile([1, 1], F32, tag="s3")
            s4 = pool.tile([1, 1], F32, tag="s4")

            cin = dpool.tile([C, B], F32, tag="cin")
            cout = dpool.tile([C, B], F32, tag="cout")
            fb = dpool.tile([1, C], F32, tag="fb")
            zt = dpool.tile([1, HID * NB], F32, tag="zt")
            gb = dpool.tile([1, C + G], F32, tag="gb")
            wb = dpool.tile([1, NB], F32, tag="wb")

            v = nc.vector
            sc = nc.scalar

            # ---- loads ----
            o0, o1 = 3 * PLANE, (3 + DH) * PLANE
            mid = 3 * PLANE + (DH // 2) * PLANE
            nc.sync.dma_start(out=xbuf[:, XG + o0 : XG + mid], in_=xin[:, o0:mid])
            nc.sync.dma_start(out=xbuf[:, XG + mid : XG + o1], in_=xin[:, mid:o1])
            nc.sync.dma_start(out=xbuf[:, XG : XG + o0], in_=xin[:, :o0])
            nc.sync.dma_start(out=xbuf[:, XG + o1 : XG + NPL * PLANE], in_=xin[:, o1:])
            nc.sync.dma_start(out=cwt[:, :], in_=cwt_in[:, :])
            nc.sync.dma_start(out=cbt[:, :], in_=cbt_in[:, :])
            nc.sync.dma_start(out=w1t[:, :], in_=w1t_in[:, :])
            nc.sync.dma_start(out=b1c[:, :], in_=b1_in[:, None])
            nc.sync.dma_start(out=w2t[:, :], in_=w2_in[:, :])
            nc.sync.dma_start(out=b2r[:, :], in_=b2_in[None, :])
            nc.sync.dma_start(out=lng[:, :], in_=lng_in[None, :])
            nc.sync.dma_start(out=lnb[:, :], in_=lnb_in[None, :])
            nc.sync.dma_start(out=onehot_bc[:, :], in_=oh4_in[:, :])
            nc.sync.dma_start(out=idp[:, :], in_=id_in[:, :])
            nc.sync.dma_start(out=g_row[:, C:], in_=gdin[None, :])

            xflat_r = xbuf[:, :]                       # fp32r view (PE rhs)
            xflat = xbuf[:, :].bitcast(F32)            # f32 view (DVE)
            xv = xflat[:, XG : XG + NPL * PLANE].rearrange(
                "c (d h w) -> c d h w", d=NPL, h=H, w=W
            )

            # ---- global-pool partial over owned planes [3, 3+DH) ----
            v.reduce_sum(featp[:, :], xv[:, 3 : 3 + DH // 2], axis=mybir.AxisListType.XYZ)
            v.reduce_sum(tmpb[:, 0:1], xv[:, 3 + DH // 2 : 3 + DH], axis=mybir.AxisListType.XYZ)
            v.tensor_tensor(out=featp[:, :], in0=featp[:, :], in1=tmpb[:, 0:1], op=ALU.add)
            v.tensor_scalar_mul(featp[:, :], featp[:, :], 1.0 / (D * HW))
            v.tensor_scalar(
                out=contrib[:, :], in0=onehot_bc[:, :], scalar1=featp[:, :],
                scalar2=None, op0=ALU.mult,
            )

            # ---- cross-core AllReduce of [C, B] partials ----
            nc.sync.dma_start(out=cin[:, :], in_=contrib[:, :])
            nc.gpsimd.collective_compute(
                "AllReduce",
                ALU.add,
                replica_groups=[list(range(N_CORES))],
                ins=[cin.opt()],
                outs=[cout.opt()],
            )
            nc.sync.dma_start(out=ar_s[:, :], in_=cout[:, :])
            v.tensor_tensor(out=ar_s[:, :], in0=ar_s[:, :], in1=onehot_bc[:, :], op=ALU.mult)
            v.reduce_sum(feat_full[:, :], ar_s[:, :], axis=mybir.AxisListType.X)

            # ---- bounce feat to a single-partition row, build g=[feat|guidance]
            nc.sync.dma_start(out=fb[:, :], in_=feat_full[:, :])
            nc.sync.dma_start(out=g_row[:, :C], in_=fb[:, :])

            # ---- LayerNorm over 192 on one partition ----
            v.reduce_sum(s1[:, :], g_row[:, :], axis=mybir.AxisListType.X)
            v.tensor_scalar_mul(s1[:, :], s1[:, :], 1.0 / (C + G))  # mu
            v.tensor_scalar(
                out=gd_row[:, :], in0=g_row[:, :], scalar1=s1[:, :], scalar2=None,
                op0=ALU.subtract,
            )
            v.tensor_tensor(out=gn_row[:, :], in0=gd_row[:, :], in1=gd_row[:, :], op=ALU.mult)
            v.reduce_sum(s2[:, :], gn_row[:, :], axis=mybir.AxisListType.X)
            v.tensor_scalar(
                out=s2[:, :], in0=s2[:, :], scalar1=1.0 / (C + G), scalar2=LN_EPS,
                op0=ALU.mult, op1=ALU.add,
            )  # var + eps
            sc.activation(s3[:, :], s2[:, :], ACTF.Sqrt)
            # one Newton step: s4 = 0.5*(s3 + (var+eps)/s3) for a clean sqrt
            v.reciprocal(s4[:, :], s3[:, :])
            v.tensor_tensor(out=s4[:, :], in0=s4[:, :], in1=s2[:, :], op=ALU.mult)
            v.tensor_tensor(out=s4[:, :], in0=s4[:, :], in1=s3[:, :], op=ALU.add)
            v.tensor_scalar_mul(s4[:, :], s4[:, :], 0.5)
            v.reciprocal(s3[:, :], s4[:, :])  # rstd
            v.tensor_scalar(
                out=gn_row[:, :], in0=gd_row[:, :], scalar1=s3[:, :], scalar2=None,
                op0=ALU.mult,
            )
            v.tensor_tensor(out=gn_row[:, :], in0=gn_row[:, :], in1=lng[:, :], op=ALU.mult)
            v.tensor_tensor(out=gn_row[:, :], in0=gn_row[:, :], in1=lnb[:, :], op=ALU.add)

            # ---- MLP layer 1: h = gelu(gn @ w1 + b1) via row-products ----
            nc.sync.dma_start(out=gb[:, :], in_=gn_row[:, :])
            nc.sync.dma_start(out=gn_bc[:, :], in_=gb[:1, :].partition_broadcast(HID))
            v.tensor_tensor(out=prod[:, :], in0=w1t[:, :], in1=gn_bc[:, :], op=ALU.mult)
            v.reduce_sum(hvec[:, :], prod[:, :], axis=mybir.AxisListType.X)
            v.tensor_tensor(out=hvec[:, :], in0=hvec[:, :], in1=b1c[:, :], op=ALU.add)
            sc.activation(hvec[:, :], hvec[:, :], ACTF.Gelu)

            # ---- MLP layer 2 via DRAM transpose bounce ----
            v.tensor_scalar(
                out=l2tmp[:, :], in0=w2t[:, :], scalar1=hvec[:, :], scalar2=None,
                op0=ALU.mult,
            )
            nc.sync.dma_start(out=zt[:, :], in_=l2tmp[:, :])
            nc.sync.dma_start(out=z72[:, :], in_=zt[:, :])
            z3 = z72[:, :].rearrange("a (j i) -> a j i", j=HID, i=NB)
            for i in range(NB):
                v.reduce_sum(zrow[:, i : i + 1], z3[:, :, i], axis=mybir.AxisListType.X)
            v.tensor_tensor(out=zrow[:, :], in0=zrow[:, :], in1=b2r[:, :], op=ALU.add)

            # ---- softmax over 3 ----
            v.reduce_max(s1[:, :], zrow[:, :], axis=mybir.AxisListType.X)
            v.tensor_scalar(
                out=zrow[:, :], in0=zrow[:, :], scalar1=s1[:, :], scalar2=None,
                op0=ALU.subtract,
            )
            sc.activation(zrow[:, :], zrow[:, :], ACTF.Exp)
            v.reduce_sum(s2[:, :], zrow[:, :], axis=mybir.AxisListType.X)
            v.reciprocal(s2[:, :], s2[:, :])
            v.tensor_scalar(
                out=wts[:, :], in0=zrow[:, :], scalar1=s2[:, :], scalar2=None,
                op0=ALU.mult,
            )

            # ---- fold gate weights into per-tap channel weights ----
            nc.sync.dma_start(out=wb[:, :], in_=wts[:, :])
            nc.sync.dma_start(out=wts_bc[:, :], in_=wb[:1, :].partition_broadcast(C))
            for i in range(NB):
                v.tensor_scalar(
                    out=w_eff[:, i * 27 : (i + 1) * 27],
                    in0=cwt[:, i * 27 : (i + 1) * 27],
                    scalar1=wts_bc[:, i : i + 1],
                    scalar2=None,
                    op0=ALU.mult,
                )
            v.tensor_scalar_mul(w_neg[:, :], w_eff[:, :], -1.0)
            v.tensor_tensor(out=tmpb[:, :], in0=cbt[:, :], in1=wts_bc[:, :], op=ALU.mult)
            v.reduce_sum(b_eff[:, :], tmpb[:, :], axis=mybir.AxisListType.X)

            # ---- the conv ----
            taps = _tap_list()
            dve_taps = taps[:DVE_TAPS]
            gp_taps = taps[DVE_TAPS : DVE_TAPS + GP_TAPS]
            pe_taps = taps[DVE_TAPS + GP_TAPS :]
            accv = acc[:, :].rearrange("c (h w) -> c h w", h=H, w=W)
            for p in range(3, 3 + DH):
                # PE side: ScalarE zeroes psum chunks and rebuilds each tap's
                # diagonal; TensorE runs flat fp32r matmuls per 448-col chunk.
                pss = []
                for ci in range(N_CHUNKS):
                    ps = ppool.tile([C, CHUNK], F32, tag=f"ps{ci}")
                    sc.memzero(ps[:, :])
                    pss.append(ps)
                for t, od, oh, ow in pe_taps:
                    dg = diagpool.tile([C, C], F32R, tag="diag")
                    sc.activation(dg[:, :], idp[:, :], ACTF.Copy,
                                  scale=w_eff[:, t : t + 1])
                    lo_trim = max(0, -oh) * W
                    hi_trim = (H - max(0, oh)) * W
                    for ci in range(N_CHUNKS):
                        a = max(ci * CHUNK, lo_trim)
                        b = min((ci + 1) * CHUNK, hi_trim)
                        src0 = XG + (p + od) * PLANE + a + oh * W + ow
                        nc.tensor.matmul(
                            pss[ci][:, a - ci * CHUNK : b - ci * CHUNK],
                            dg[:, :],
                            xflat_r[:, src0 : src0 + (b - a)],
                            start=False,
                            stop=False,
                            skip_group_check=True,
                        )
                # DVE side: exact fp32 MACs (on the rounded x)
                for n, (t, od, oh, ow) in enumerate(dve_taps):
                    h0i, h1i = max(0, oh), H + min(0, oh)
                    w0i, w1i = max(0, ow), W + min(0, ow)
                    h0o, h1o = max(0, -oh), H + min(0, -oh)
                    w0o, w1o = max(0, -ow), W + min(0, -ow)
                    in_ap = xv[:, p + od, h0i:h1i, w0i:w1i]
                    out_ap = accv[:, h0o:h1o, w0o:w1o]
                    if n == 0:
                        # full-extent center tap initializes acc with bias
                        v.tensor_scalar(
                            out=out_ap, in0=in_ap, scalar1=w_eff[:, t : t + 1],
                            scalar2=b_eff[:, :], op0=ALU.mult, op1=ALU.add,
                        )
                    else:
                        v.scalar_tensor_tensor(
                            out=out_ap, in0=in_ap, scalar=w_eff[:, t : t + 1],
                            in1=out_ap, op0=ALU.mult, op1=ALU.add,
                        )
                # GPSIMD side: extra taps into a separate accumulator
                for n, (t, od, oh, ow) in enumerate(gp_taps):
                    h0i, h1i = max(0, oh), H + min(0, oh)
                    w0i, w1i = max(0, ow), W + min(0, ow)
                    h0o, h1o = max(0, -oh), H + min(0, -oh)
                    w0o, w1o = max(0, -ow), W + min(0, -ow)
                    in_ap = xv[:, p + od, h0i:h1i, w0i:w1i]
                    out_ap = acc2[:, :].rearrange(
                        "c (h w) -> c h w", h=H, w=W
                    )[:, h0o:h1o, w0o:w1o]
                    if n == 0:
                        nc.gpsimd.tensor_scalar(
                            out=acc2[:, :], in0=acc2[:, :], scalar1=0.0,
                            scalar2=None, op0=ALU.mult,
                        )
                        nc.gpsimd.scalar_tensor_tensor(
                            out=out_ap, in0=in_ap, scalar=w_eff[:, t : t + 1],
                            in1=out_ap, op0=ALU.mult, op1=ALU.add,
                        )
                    else:
                        nc.gpsimd.scalar_tensor_tensor(
                            out=out_ap, in0=in_ap, scalar=w_eff[:, t : t + 1],
                            in1=out_ap, op0=ALU.mult, op1=ALU.add,
                        )
                if gp_taps:
                    v.tensor_tensor(out=acc[:, :], in0=acc[:, :], in1=acc2[:, :],
                                    op=ALU.add)
                # merge PSUM chunks into acc
                for ci in range(N_CHUNKS):
                    seg = slice(ci * CHUNK, (ci + 1) * CHUNK)
                    v.tensor_tensor(
                        out=acc[:, seg], in0=acc[:, seg], in1=pss[ci][:, :],
                        op=ALU.add,
                    )
                # fix up the w-wrap columns the flat PE shifts got wrong
                for t, od, oh, ow in pe_taps:
                    if ow == 0:
                        continue
                    r0 = max(0, -oh)
                    nr = H - abs(oh)
                    w0 = W - ow if ow > 0 else 0
                    nw = abs(ow)
                    base = XG + (p + od) * PLANE + (r0 + oh) * W + (w0 + ow)
                    src = xflat[:, base : base + nr * W].rearrange(
                        "c (r w) -> c r w", r=nr, w=W
                    )[:, :, 0:nw]
                    out2d = accv[:, r0 : r0 + nr, w0 : w0 + nw]
                    v.scalar_tensor_tensor(
                        out=out2d, in0=src, scalar=w_neg[:, t : t + 1],
                        in1=out2d, op0=ALU.mult, op1=ALU.add,
                    )
                nc.sync.dma_start(
                    out=yout[:, (p - 3) * PLANE : (p - 2) * PLANE], in_=acc[:, :]
                )

    _split_sem_waits(nc)
    return nc


_WAITSPLIT = [0]


def _split_sem_waits(nc, max_waits=1):
    """This walrus build rejects >1 SyncWait per instruction (and any wait on
    a Drain). Move excess waits onto same-engine NOPs inserted just before."""
    for bb in nc.main_func.blocks:
        insns = bb.instructions
        i = 0
        while i < len(insns):
            ins = insns[i]
            si = ins.sync_info
            limit = 0 if ins.opcode == "Drain" else max_waits
            if si is not None and si.on_wait is not None and len(si.on_wait) > limit:
                waits = list(si.on_wait)
                keep = waits[-limit:] if limit else []
                extra = waits[: len(waits) - limit]
                pos = i
                for j in range(0, len(extra), max_waits):
                    nop = mybir.InstNoOp(
                        name=f"I-waitsplit-{_WAITSPLIT[0]}", ins=[], outs=[]
                    )
                    _WAITSPLIT[0] += 1
                    nop.engine = ins.engine
                    nop.sync_info = mybir.SyncInfo(
                        on_wait=extra[j : j + max_waits], on_update=[]
                    )
                    insns.insert(pos, nop)
                    pos += 1
                    i += 1
                si.on_wait = keep
            i += 1


def _round_fp32r(a):
    u = np.ascontiguousarray(a, dtype=np.float32).view(np.uint32)
    lsb = (u >> 12) & 1
    r = ((u + 0x7FF + lsb) & np.uint32(0xFFFFF000)).astype(np.uint32)
    return r.view(np.float32)


def _prep_inputs(x, guidance, convw, convb, ln_g, ln_b, w1, b1, w2, b2):
    f = np.float32
    cwt = np.ascontiguousarray(
        convw.reshape(NB, C, 27).transpose(1, 0, 2).reshape(C, NB * 27), dtype=f
    )
    cbt = np.ascontiguousarray(convb.T, dtype=f)
    w1t = np.ascontiguousarray(w1.T, dtype=f)
    idp = np.eye(C, dtype=f)
    common = dict(
        cwt=cwt, cbt=cbt, w1t=w1t,
        b1=np.ascontiguousarray(b1, dtype=f),
        w2=np.ascontiguousarray(w2, dtype=f),
        b2=np.ascontiguousarray(b2, dtype=f),
        lng=np.ascontiguousarray(ln_g, dtype=f),
        lnb=np.ascontiguousarray(ln_b, dtype=f),
        idp=idp,
    )
    in_maps = []
    for core in range(N_CORES):
        b, h = core // 2, core % 2
        lo = 8 * h - 3
        shard = np.zeros((C, NPL, H, W), dtype=f)
        g0, g1 = max(0, lo), min(D, lo + NPL)
        shard[:, g0 - lo : g1 - lo] = x[b, :, g0:g1]
        onehot = np.zeros((C, B), dtype=f)
        onehot[:, b] = 1.0
        in_maps.append(
            dict(
                x=_round_fp32r(shard.reshape(C, NPL * PLANE)),
                gd=np.ascontiguousarray(guidance[b], dtype=f),
                oh4=onehot,
                **common,
            )
        )
    return in_maps


_CACHED_NC = None


def kernel(x, guidance, convw, convb, ln_g, ln_b, w1, b1, w2, b2):
    global _CACHED_NC
    if _CACHED_NC is None:
        _CACHED_NC = _build_program()
    in_maps = _prep_inputs(
        x, guidance, convw, convb, ln_g, ln_b, w1, b1, w2, b2
    )
    res = run_bass_kernel_spmd(_CACHED_NC, in_maps, list(range(N_CORES)))
    out = np.empty((B, C, D, H, W), dtype=np.float32)
    for core in range(N_CORES):
        b, h = core // 2, core % 2
        out[b, :, 8 * h : 8 * h + 8] = res.results[core]["y"].reshape(C, DH, H, W)
    return out


if __name__ == "__main__":
    rng = np.random.default_rng(0)
    ins = dict(
        x=rng.standard_normal((B, C, D, H, W), dtype=np.float32),
        guidance=rng.standard_normal((B, G), dtype=np.float32),
        convw=(rng.standard_normal((NB, C, 1, K, K, K)) * 0.1).astype(np.float32),
        convb=np.zeros((NB, C), np.float32),
        ln_g=np.ones((C + G,), np.float32),
        ln_b=np.zeros((C + G,), np.float32),
        w1=(rng.standard_normal((C + G, HID)) * 0.05).astype(np.float32),
        b1=np.zeros((HID,), np.float32),
        w2=(rng.standard_normal((HID, NB)) * 0.05).astype(np.float32),
        b2=np.zeros((NB,), np.float32),
    )
    out = kernel(**ins)
    print("kernel ran, out shape", out.shape, "mean", float(np.abs(out).mean()))
